# revision 1
# baseline (speedup 1.0000x reference)
# Tensor-parallel fused attention kernel for TRN2, 8 cores — v2.
#
# Core r owns heads {2r, 2r+1} for QKV+attention; the out-projection is
# resharded by tokens via AllToAll (core r owns tokens [256r, 256r+256) of
# each batch and computes all 1024 output dims for them).
#
# Host-side preprocessing folded into the weights:
#  - q/k head blocks of w_qkv are mean-centered per head (LayerNorm mean
#    subtract == matmul against col-centered weights)
#  - q/k head blocks are de-interleaved (evens then odds) so RoPE reads are
#    stride-1 (DVE 2x f16); q.k dot products are permutation invariant
#  - cos/sin tables precomputed on host (f16)
#  - x pre-cast to f16
#
# Device pipeline per batch: per 128-token tile T: x load + DMA transpose,
# 8 QKV matmuls -> PSUM; DVE: sum-sq stats, evac q*rstd_q / k*rstd_k to f16,
# v copy (zero-padded per head); ACT: ln/exp rstd, exp(-gate); rope on DVE
# (f16 2x); batched DMA transpose to qkgT.  Attention per (b, Q-tile of 512):
# per k-tile x head: scores MM (64-contraction), exp on ACT (f16 out),
# PV MM (zero-padded v => full-bank accumulate), denominator via ones-MM
# (or DVE accumulate, see dve_denom); reciprocal + PE broadcast; gate mult.
# og chunks -> AllToAll per batch -> out-projection for own token slice.
import math

import concourse.bass as bass
import concourse.mybir as mybir
from concourse import bacc, tile


def _install_act_table_patch():
    """Steer Exp/Ln/Copy activations to the one table set containing all
    three (natural_log_exp_and_others), so the kernel needs a single
    ACT_TABLE_LOAD instead of thrashing between exp_and_others and
    natural_log per tile.  Set indices are preserved (entries are masked,
    not reordered), so the emitted act_func_set_id stays consistent with
    act_info.json for walrus/NRT."""
    import concourse.hw_specs as hw_specs
    import concourse.bass_interp as bass_interp

    if getattr(hw_specs, "_attn_v2_table_patch", False):
        return
    orig = hw_specs.get_activation_tables
    AF_ = mybir.ActivationFunctionType
    steer = {AF_.Exp, AF_.Ln, AF_.Copy}
    target = "natural_log_exp_and_others"

    def patched(arch):
        tables = orig(arch)
        if target not in tables:
            return tables
        return {
            name: (funcs if name == target else set(funcs) - steer)
            for name, funcs in tables.items()
        }

    hw_specs._attn_v2_table_patch = True
    hw_specs.get_activation_tables = patched
    bacc.get_activation_tables = patched
    bass_interp.get_activation_tables = patched


_install_act_table_patch()

F32 = mybir.dt.float32
F16 = mybir.dt.float16
AF = mybir.ActivationFunctionType
ALU = mybir.AluOpType
AX = mybir.AxisListType

DIM = 1024
HD = 64
EPS = 1e-5


def build(S: int, n_cores: int = 8, reps: int = 1, apply_ln_affine: bool = False,
          dve_denom: bool = True, gp_denom: bool = False):
    TB = S // 128             # 128-token tiles per batch
    TT = 2 * TB
    QW = 512                  # q tokens per attention chunk
    QT = S // QW              # q chunks per batch (4)
    NTOK = 2 * S // n_cores   # output tokens owned per core (512)
    TSLICE = S // n_cores     # tokens per batch per core (256)

    nc = bacc.Bacc("TRN2", target_bir_lowering=False, debug=False, num_devices=n_cores)

    XT = nc.dram_tensor("x16t", [DIM, 2 * S], F16, kind="ExternalInput")
    WQKV = nc.dram_tensor("wqkv", [DIM, 512], F16, kind="ExternalInput")
    WOUT = nc.dram_tensor("wout", [DIM, DIM], F16, kind="ExternalInput")
    COS = nc.dram_tensor("cos16", [S, 32], F16, kind="ExternalInput")
    SIN = nc.dram_tensor("sin16", [S, 32], F16, kind="ExternalInput")
    LNP = nc.dram_tensor("lnp", [8, HD], F32, kind="ExternalInput")
    SELCB = nc.dram_tensor("selcb", [128, 2, 128], F16, kind="ExternalInput")
    OUT = nc.dram_tensor("out", [NTOK, DIM], F16, kind="ExternalOutput")

    HSL = TSLICE // 2         # tokens per half-batch A2A slice (128)
    a2a_in = [nc.dram_tensor(f"a2a_in{c}", [n_cores, 128, HSL], F16)
              for c in range(4)]
    a2a_out = [nc.dram_tensor(f"a2a_out{c}", [n_cores, 128, HSL], F16)
               for c in range(4)]

    with tile.TileContext(nc) as tc:
        with (
            tc.tile_pool(name="persist", bufs=1) as pp,
            tc.tile_pool(name="work", bufs=2) as wp,
            tc.tile_pool(name="espool", bufs=4) as ep,
            tc.tile_pool(name="xload", bufs=3) as xp,
            tc.tile_pool(name="small", bufs=2) as sp,
        ):
            # ---- weights (scalar-queue HWDGE: pool queue stays free for
            # x loads, ACT engine is idle during the preamble) ----
            w16 = pp.tile([128, 8, 512], F16, tag="w16")
            nc.scalar.dma_start(
                w16[:, 0:1, :],
                bass.AP(WQKV.ap().tensor, 0, [[512, 128], [512 * 128, 1], [1, 512]]))
            nc.scalar.dma_start(
                w16[:, 1:8, :],
                bass.AP(WQKV.ap().tensor, 512 * 128, [[512, 128], [512 * 128, 7], [1, 512]]))
            w16o = pp.tile([128, 8, DIM], F16, tag="w16o")
            nc.scalar.dma_start(
                w16o[:],
                bass.AP(WOUT.ap().tensor, 0, [[DIM, 128], [DIM * 128, 8], [1, DIM]]))

            # cos/sin [tok-in-tile, tile-in-batch, angle]
            cosb = pp.tile([128, TB, 32], F16, tag="cosb")
            nc.scalar.dma_start(
                cosb[:], bass.AP(COS.ap().tensor, 0, [[32, 128], [128 * 32, TB], [1, 32]]))
            sinb = pp.tile([128, TB, 32], F16, tag="sinb")
            nc.scalar.dma_start(
                sinb[:], bass.AP(SIN.ap().tensor, 0, [[32, 128], [128 * 32, TB], [1, 32]]))

            # denominator block-selector [128, 2, 128] f16: face h has ones in
            # cols of head h -> the ones-matmul lands each head's denominator
            # replicated across that head's 64 output partitions
            selcb = pp.tile([128, 2, 128], F16, tag="selcb")
            nc.scalar.dma_start(selcb[:], SELCB.ap())

            if apply_ln_affine:
                lnp1 = sp.tile([1, 512], F32, tag="lnp1")
                nc.gpsimd.dma_start(
                    lnp1[:], LNP.ap().rearrange("a b -> (a b)").unsqueeze(0))
                ones1 = sp.tile([1, 128], F32, tag="ones1")
                nc.vector.memset(ones1[:], 1.0)
                with tc.tile_pool(name="pbc", bufs=1, space="PSUM") as pbc:
                    lnb_ps = pbc.tile([128, 512], F32)
                    nc.tensor.matmul(lnb_ps[:], ones1[:], lnp1[:], start=True, stop=True)
                    lnwb = pp.tile([128, 512], F32, tag="lnwb")
                    nc.scalar.copy(lnwb[:], lnb_ps[:])

            epsc = pp.tile([128, 1], F32, tag="epsc")
            nc.vector.memset(epsc[:], EPS)
            # dummy activation: pulls the one ACT table load into the startup
            # window instead of stalling the first real ln
            warm = sp.tile([128, 1], F32, tag="warm")
            nc.scalar.activation(warm[:], epsc[:], AF.Exp)

            # ---- persistent activations ----
            qkg16 = pp.tile([128, TT, 384], F16, tag="qkg16")
            qkgT = pp.tile([128, TT, 3, 128], F16, tag="qkgT")
            # v, zero-padded per head: face 0 = [v_h0 | 0], face 1 = [0 | v_h1]
            v16z = pp.tile([128, TT, 2, 128], F16, tag="v16z")
            nc.vector.memset(v16z[:], 0.0)
            ssq = pp.tile([128, TT, 4], F32, tag="ssq")
            rstd = pp.tile([128, TT, 4], F32, tag="rstd")
            og = pp.tile([128, 2, S], F16, tag="og")

            for _rep in range(reps):
              with (
                tc.tile_pool(name="ps_a", bufs=2, space="PSUM") as ps_a,
                tc.tile_pool(name="ps_s", bufs=2, space="PSUM") as ps_s,
                tc.tile_pool(name="ps_o", bufs=1, space="PSUM") as ps_o,
                tc.tile_pool(name="ps_d", bufs=1, space="PSUM") as ps_d,
              ):
                GRP = 4   # x tiles loaded per DMA (x arrives pre-transposed)
                # batch-local load-group starts: first two groups are small so
                # the first matmul can start early
                grp_starts = {0: {0: 1, 1: 3, 4: 4, 8: 4, 12: 4},
                              1: {0: 4, 4: 4, 8: 4, 12: 4}}
                cur_start = [0]

                def load_x_group(T, n):
                    nonlocal xT16
                    xT16 = xp.tile([128, 8, GRP * 128], F16, tag="xT16")
                    nc.gpsimd.dma_start(
                        xT16[:, :, 0:n * 128],
                        bass.AP(XT.ap().tensor, T * 128,
                                [[2 * S, 128], [128 * 2 * S, 8], [1, n * 128]]))

                def phase1_tile(T, b):
                    # qkv matmuls + stats + evacs for one 128-token tile
                    tb = T - b * TB
                    n = grp_starts[b].get(tb)
                    if n is not None:
                        load_x_group(T, n)
                        cur_start[0] = tb
                    g = tb - cur_start[0]
                    psq = ps_a.tile([128, 512], F32, tag="psa")
                    for c in range(8):
                        nc.tensor.matmul(psq[:],
                                         xT16[:, c, g * 128:(g + 1) * 128],
                                         w16[:, c, :],
                                         start=(c == 0), stop=(c == 7))

                    # evac q,k to f16 first; stats run from SBUF (cheaper DVE
                    # modes; a DVE tensor_tensor also may not read 2 PSUM
                    # operands), then scale by rstd in place
                    qk = qkg16[:, T, 0:256]
                    nc.vector.tensor_copy(qk, psq[:, 0:256])
                    t1 = wp.tile([128, 256], F16, tag="t1")
                    nc.vector.tensor_tensor(t1[:], qk, qk, ALU.mult)
                    nc.vector.tensor_reduce(
                        ssq[:, T, :], t1[:].rearrange("p (s d) -> p s d", d=HD),
                        AX.X, ALU.add)
                    # rstd = exp(-0.5 * ln(ssq/64 + eps))
                    nc.scalar.activation(rstd[:, T, :], ssq[:, T, :], AF.Ln,
                                         scale=1.0 / HD, bias=epsc[:])
                    nc.scalar.activation(rstd[:, T, :], rstd[:, T, :], AF.Exp,
                                         scale=-0.5)
                    nc.vector.tensor_tensor(
                        qk.rearrange("p (s d) -> p s d", d=HD),
                        qk.rearrange("p (s d) -> p s d", d=HD),
                        rstd[:, T, :].unsqueeze(2).broadcast_to([128, 4, HD]),
                        ALU.mult)
                    if apply_ln_affine:
                        nc.vector.tensor_tensor(
                            qkg16[:, T, 0:256], qkg16[:, T, 0:256],
                            lnwb[:, 0:256], ALU.mult)
                        nc.vector.tensor_tensor(
                            qkg16[:, T, 0:256], qkg16[:, T, 0:256],
                            lnwb[:, 256:512], ALU.add)
                    # v evac (zero-padded halves)
                    nc.vector.tensor_copy(v16z[:, T, 0, 0:64], psq[:, 256:320])
                    nc.vector.tensor_copy(v16z[:, T, 1, 64:128], psq[:, 320:384])
                    # gate: exp(-g)
                    nc.scalar.activation(qkg16[:, T, 256:384], psq[:, 384:512],
                                         AF.Exp, scale=-1.0)

                    # rope per 4-tile chunk (short DVE bursts), transpose per 8
                    if tb % 4 == 3:
                        T0 = T - 3
                        xt = qkg16.tensor
                        base = qkg16[:, T0, 0].offset
                        pstep = qkg16[:].ap[0][0]
                        x1 = bass.AP(xt, base, [[pstep, 128], [384, 4], [HD, 4], [1, 32]])
                        x2 = bass.AP(xt, base + 32, [[pstep, 128], [384, 4], [HD, 4], [1, 32]])
                        cstep = cosb[:].ap[0][0]
                        cosa = bass.AP(cosb.tensor, cosb[:].offset + (T0 - b * TB) * 32,
                                       [[cstep, 128], [32, 4], [0, 4], [1, 32]])
                        sina = bass.AP(sinb.tensor, sinb[:].offset + (T0 - b * TB) * 32,
                                       [[cstep, 128], [32, 4], [0, 4], [1, 32]])
                        sh = [128, 4, 4, 32]
                        ta = wp.tile(sh, F16, tag="ta")
                        tb_ = wp.tile(sh, F16, tag="tb")
                        tc_ = wp.tile(sh, F16, tag="tc")
                        td = wp.tile(sh, F16, tag="td")
                        nc.vector.tensor_tensor(ta[:], x1, cosa, ALU.mult)
                        nc.vector.tensor_tensor(tc_[:], x1, sina, ALU.mult)
                        nc.vector.tensor_tensor(tb_[:], x2, sina, ALU.mult)
                        nc.vector.tensor_tensor(td[:], x2, cosa, ALU.mult)
                        nc.vector.tensor_tensor(x1, ta[:], tb_[:], ALU.subtract)
                        nc.vector.tensor_tensor(x2, tc_[:], td[:], ALU.add)
                    if tb % 8 == 7:
                        T0 = T - 7
                        nc.sync.dma_start_transpose(
                            qkgT[:, T0:T0 + 8, :, :].rearrange("p t f c -> p (t f) c"),
                            qkg16[:, T0:T0 + 8, :].rearrange("p t d -> p (t d)"))

                def attn_chunk(b, Q):
                    # attention for one 512-token q chunk; half-batch AllToAll
                    # + out-projection after odd chunks
                    use_dve = dve_denom
                    Ts = b * TB + Q * 4
                    po = ps_o.tile([128, QW], F32, tag="po")
                    pd = ps_d.tile([128, QW], F32, tag="pd")
                    if use_dve:
                        acc16 = wp.tile([128, 2, QW], F16, tag="acc16")
                    for Jb in range(TB):
                        Tj = b * TB + Jb
                        ps = ps_s.tile([128, 2, QW], F32, tag="ps")
                        for h in range(2):
                            nc.tensor.matmul(
                                ps[:, h, :],
                                qkgT[64 * h:64 * h + 64, Tj, 1, :],
                                qkgT[64 * h:64 * h + 64, Ts:Ts + 4, 0, :],
                                start=True, stop=True)
                        es = ep.tile([128, 2, QW], F16, tag="es")
                        nc.scalar.activation(es[:], ps[:], AF.Exp, scale=0.125)
                        for h in range(2):
                            nc.tensor.matmul(
                                po[:], v16z[:, Tj, h, :], es[:, h, :],
                                start=(Jb == 0 and h == 0),
                                stop=(Jb == TB - 1 and h == 1),
                                skip_group_check=True)
                        if use_dve:
                            eng = nc.gpsimd if gp_denom else nc.vector
                            if Jb == 0:
                                eng.tensor_copy(acc16[:], es[:])
                            else:
                                eng.tensor_tensor(acc16[:], acc16[:], es[:],
                                                  ALU.add)
                        else:
                            for h in range(2):
                                nc.tensor.matmul(
                                    pd[:], selcb[:, h, :], es[:, h, :],
                                    start=(Jb == 0 and h == 0),
                                    stop=(Jb == TB - 1 and h == 1),
                                    skip_group_check=True)
                    if use_dve:
                        for h in range(2):
                            nc.tensor.matmul(pd[:], selcb[:, h, :],
                                             acc16[:, h, :],
                                             start=(h == 0), stop=(h == 1),
                                             skip_group_check=True)
                    r32 = wp.tile([128, QW], F32, tag="r32")
                    nc.vector.reciprocal(r32[:], pd[:])
                    on16 = wp.tile([128, QW], F16, tag="on16")
                    nc.vector.tensor_tensor(on16[:], po[:], r32[:], ALU.mult)
                    # sigmoid(g) = 1 / (1 + exp(-g)), exp(-g) rode the transpose
                    sg = wp.tile([128, 4, 128], F16, tag="sg")
                    nc.vector.tensor_scalar_add(sg[:], qkgT[:, Ts:Ts + 4, 2, :], 1.0)
                    with nc.allow_low_precision(reason="sigmoid in (0,1), f16 ok"):
                        nc.vector.reciprocal(sg[:], sg[:])
                    nc.vector.tensor_tensor(
                        og[:, b, Q * QW:(Q + 1) * QW].rearrange(
                            "p (a c) -> p a c", c=128),
                        on16[:].rearrange("p (a c) -> p a c", c=128),
                        sg[:], ALU.mult)

                    # ---- phase 3: half-batch AllToAll + out-projection ----
                    if Q % 2 == 1:
                        half = Q // 2
                        ch = 2 * b + half
                        nc.gpsimd.dma_start(
                            bass.AP(a2a_in[ch].ap().tensor, 0,
                                    [[HSL, 128], [128 * HSL, n_cores], [1, HSL]]),
                            og[:, b, half * 1024:(half + 1) * 1024].rearrange(
                                "p (c t) -> p c t", t=HSL))
                        nc.gpsimd.collective_compute(
                            "AllToAll", ALU.bypass,
                            replica_groups=[list(range(n_cores))],
                            ins=[a2a_in[ch].ap()], outs=[a2a_out[ch].ap()],
                        )
                        oga = sp.tile([128, 8, HSL], F16, tag="oga")
                        nc.gpsimd.dma_start(
                            oga[:],
                            bass.AP(a2a_out[ch].ap().tensor, 0,
                                    [[HSL, 128], [128 * HSL, n_cores], [1, HSL]]))
                        ot16 = wp.tile([128, 2, 512], F16, tag="ot16")
                        for jb in range(DIM // 512):
                            pot = ps_a.tile([128, 512], F32, tag="psa")
                            for c in range(8):
                                nc.tensor.matmul(
                                    pot[:], oga[:, c, :],
                                    w16o[:, c, jb * 512:(jb + 1) * 512],
                                    start=(c == 0), stop=(c == 7))
                            nc.vector.tensor_copy(ot16[:, jb, :], pot[:])
                        nc.scalar.dma_start(
                            bass.AP(OUT.ap().tensor, ch * HSL * DIM,
                                    [[DIM, 128], [1, DIM]]),
                            ot16[:].rearrange("p a b -> p (a b)"))

                # ---- emission order: interleave phase1(b1) tiles between
                # attn(b0) chunks so the scheduler keeps both pipelines fed ----
                xT16 = None
                for T in range(0, TT):
                    phase1_tile(T, T // TB)
                for Q in range(QT):
                    attn_chunk(0, Q)
                for Q in range(QT):
                    attn_chunk(1, Q)

    nc.compile()
    return nc


def _prep_wqkv(w_qkv, r):
    """Per-core wqkv slice with q/k centered + de-interleaved."""
    import numpy as np
    cols = []
    perm = np.concatenate([np.arange(0, HD, 2), np.arange(1, HD, 2)])
    for sec in range(4):
        blk = np.array(w_qkv[:, sec * DIM + 128 * r: sec * DIM + 128 * r + 128])
        if sec < 2:  # q, k: center + de-interleave per head
            for h in range(2):
                hb = blk[:, h * HD:(h + 1) * HD]
                hb = hb - hb.mean(axis=1, keepdims=True)
                blk[:, h * HD:(h + 1) * HD] = hb[:, perm]
        cols.append(blk)
    return np.ascontiguousarray(np.concatenate(cols, axis=1), dtype=np.float16)


def shard_inputs(x, freqs, w_qkv, w_out, qn_w, qn_b, kn_w, kn_b, n_cores=8):
    import numpy as np
    B, S, _ = x.shape
    x16 = np.ascontiguousarray(x.reshape(2 * S, DIM), dtype=np.float16)
    cos16 = np.cos(np.asarray(freqs)).astype(np.float16)
    sin16 = np.sin(np.asarray(freqs)).astype(np.float16)
    wo16 = np.ascontiguousarray(w_out, dtype=np.float16)
    perm = np.concatenate([np.arange(0, HD, 2), np.arange(1, HD, 2)])
    lnp_base = np.stack([qn_w[perm], qn_w[perm], kn_w[perm], kn_w[perm],
                         qn_b[perm], qn_b[perm], kn_b[perm], kn_b[perm]]).astype(np.float32)
    selcb = np.zeros((128, 2, 128), np.float16)
    selcb[:, 0, 0:64] = 1.0
    selcb[:, 1, 64:128] = 1.0
    x16t = np.ascontiguousarray(x16.T)
    maps = []
    for r in range(n_cores):
        maps.append({
            "x16t": x16t,
            "wqkv": _prep_wqkv(w_qkv, r),
            "wout": wo16,
            "cos16": cos16, "sin16": sin16,
            "lnp": lnp_base,
            "selcb": selcb,
        })
    return maps


def unshard_output(results, S, n_cores=8):
    import numpy as np
    HSL = S // n_cores // 2
    out = np.empty((2, S, DIM), np.float32)
    for r, res in enumerate(results):
        o = np.asarray(res["out"], dtype=np.float32)  # [4*HSL, DIM]
        for b in range(2):
            for half in range(2):
                ch = 2 * b + half
                t0 = half * (S // 2) + r * HSL
                out[b, t0:t0 + HSL] = o[ch * HSL:(ch + 1) * HSL]
    return out


_NC_CACHE = {}


def _get_nc(S, affine):
    key = (S, affine)
    if key not in _NC_CACHE:
        _NC_CACHE[key] = build(S, apply_ln_affine=affine)
    return _NC_CACHE[key]


def kernel(x, freqs, w_qkv, w_out, qn_w, qn_b, kn_w, kn_b):
    """Full-input entrypoint: shards across 8 neuron cores, runs, gathers."""
    import numpy as np
    from concourse.bass_utils import run_bass_kernel_spmd

    x = np.asarray(x, dtype=np.float32)
    freqs = np.asarray(freqs, dtype=np.float32)
    w_qkv = np.asarray(w_qkv, dtype=np.float32)
    w_out = np.asarray(w_out, dtype=np.float32)
    qn_w, qn_b = np.asarray(qn_w), np.asarray(qn_b)
    kn_w, kn_b = np.asarray(kn_w), np.asarray(kn_b)
    B, S, _ = x.shape
    affine = not (np.all(qn_w == 1) and np.all(qn_b == 0)
                  and np.all(kn_w == 1) and np.all(kn_b == 0))
    nc = _get_nc(S, bool(affine))
    maps = shard_inputs(x, freqs, w_qkv, w_out, qn_w, qn_b, kn_w, kn_b)
    res = run_bass_kernel_spmd(nc, maps, list(range(8)))
    return unshard_output(res.results, S)

